# revision 2
# baseline (speedup 1.0000x reference)
"""Multi-head attention (B=2, S=2048, D=1024, H=16) on 8 Trainium2 NeuronCores.

Sharding: batch x head-group. Core c handles batch b = c//4 and heads
[4*(c%4), 4*(c%4)+4) (a 256-wide slice of the QKV projection output and the
matching 256-row slice of Wo). Each core computes its partial output
projection; a 4-way ReduceScatter per batch group sums the partials and
leaves each core with a [512, 1024] row block of the final output, which the
host reassembles.

Per-core dataflow (all matmul operands fp16, fp32 PSUM accumulation):
  - x^T tiles arrive via hardware DMA-transpose (fp16).
  - Q^T, K^T feature-major [256, 2048]; V token-major [2048, 256].
  - Scores computed transposed (S^T[k, q] = K_h @ Q_h^T) with two heads
    row-packed in the PE array; softmax without max-subtraction (exp via
    ScalarE with the 1/sqrt(dh) scale folded in); per-q sums via a
    ones-matmul col-packed two heads at a time, broadcast across partitions
    for free; attn@V col-packed two heads at a time.
  - Output projection from attn_norm^T with the head-pair dim as the
    contraction tiles.
"""

import numpy as np

import concourse.bass as bass  # noqa: F401  (engine namespaces via nc)
import concourse.mybir as mybir
import concourse.tile as tile
from concourse import bacc
from concourse.bass_utils import run_bass_kernel_spmd

F32 = mybir.dt.float32
F16 = mybir.dt.float16
AF = mybir.ActivationFunctionType

B, S, D = 2, 2048, 1024
H, DH = 16, 64
NCORES = 8
GPB = 4                # cores per batch group
HPC = H // GPB         # heads per core
DS = HPC * DH          # 256: per-core slice of the projection output
P = 128
NDT = D // P           # 8 d_model tiles
NTT = S // P           # 16 token tiles
QCH = 512              # q-chunk (PSUM bank = 512 fp32)
NQC = S // QCH         # 4
NKT = S // P           # 16 k tiles
SCALE = float(1.0 / np.sqrt(DH))

REPLICA_GROUPS = [[0, 1, 2, 3], [4, 5, 6, 7]]

_CACHED_NC = None


def _build_module():
    nc = bacc.Bacc("TRN2", target_bir_lowering=False, debug=False,
                   num_devices=NCORES)

    xq_d = nc.dram_tensor("xq", [S, D], F16, kind="ExternalInput")
    xk_d = nc.dram_tensor("xk", [S, D], F16, kind="ExternalInput")
    xv_d = nc.dram_tensor("xv", [S, D], F16, kind="ExternalInput")
    wq_d = nc.dram_tensor("wq", [D, DS], F16, kind="ExternalInput")
    wk_d = nc.dram_tensor("wk", [D, DS], F16, kind="ExternalInput")
    wv_d = nc.dram_tensor("wv", [D, DS], F16, kind="ExternalInput")
    wo_d = nc.dram_tensor("wo", [DS, D], F16, kind="ExternalInput")
    bq_d = nc.dram_tensor("bq", [DS, 1], F32, kind="ExternalInput")
    bk_d = nc.dram_tensor("bk", [DS, 1], F32, kind="ExternalInput")
    bv_d = nc.dram_tensor("bv", [1, DS], F32, kind="ExternalInput")
    bo_d = nc.dram_tensor("bo", [1, D], F32, kind="ExternalInput")

    out_d = nc.dram_tensor("out", [S // GPB, D], F32, kind="ExternalOutput")
    partial_d = nc.dram_tensor("partial", [S, D], F32)
    rs_d = nc.dram_tensor("rs_out", [S // GPB, D], F32)

    with tile.TileContext(nc) as tc:
        with (
            tc.tile_pool(name="cst", bufs=1) as cst,
            tc.tile_pool(name="xt", bufs=10) as xtp,
            tc.tile_pool(name="exp", bufs=4) as expp,
            tc.tile_pool(name="rcp", bufs=2) as rcpp,
            tc.tile_pool(name="osb", bufs=2) as osbp,
            tc.tile_pool(name="psA", bufs=2, space="PSUM") as psA,
            tc.tile_pool(name="psB", bufs=2, space="PSUM") as psB,
        ):
            # ---- constants ----
            wq_t = cst.tile([P, NDT, DS], F16, tag="wq")
            wk_t = cst.tile([P, NDT, DS], F16, tag="wk")
            wv_t = cst.tile([P, NDT, DS], F16, tag="wv")
            wo_t = cst.tile([P, 2, D], F16, tag="wo")
            nc.sync.dma_start(wq_t[:], wq_d.rearrange("(a p) n -> p a n", p=P))
            nc.sync.dma_start(wk_t[:], wk_d.rearrange("(a p) n -> p a n", p=P))
            nc.sync.dma_start(wv_t[:], wv_d.rearrange("(a p) n -> p a n", p=P))
            nc.sync.dma_start(wo_t[:], wo_d.rearrange("(a p) n -> p a n", p=P))

            bq_t = cst.tile([P, 2, 1], F32, tag="bq")
            bk_t = cst.tile([P, 2, 1], F32, tag="bk")
            nc.sync.dma_start(bq_t[:], bq_d.rearrange("(a p) o -> p a o", p=P))
            nc.sync.dma_start(bk_t[:], bk_d.rearrange("(a p) o -> p a o", p=P))

            bv_row = cst.tile([1, DS], F32, tag="bvr")
            bo_row = cst.tile([1, D], F32, tag="bor")
            nc.sync.dma_start(bv_row[:], bv_d[:])
            nc.sync.dma_start(bo_row[:], bo_d[:])
            bv_b = cst.tile([P, DS], F32, tag="bvb")
            bo_b = cst.tile([P, D], F32, tag="bob")
            nc.gpsimd.partition_broadcast(bv_b[:], bv_row[:])
            nc.gpsimd.partition_broadcast(bo_b[:], bo_row[:])

            ones_t = cst.tile([P, DH], F16, tag="ones")
            nc.vector.memset(ones_t[:], 1.0)

            # ---- activations: resident projections ----
            qt_t = cst.tile([P, 2, S], F16, tag="qt")   # Q^T  (pair, t)
            kt_t = cst.tile([P, 2, S], F16, tag="kt")   # K^T
            v_t = cst.tile([P, NTT, DS], F16, tag="vt")  # V token-major
            an_t = cst.tile([P, 2, S], F16, tag="an")   # attn_norm^T

            # ---- transposed input tiles (DMA transpose, fp16) ----
            def load_xt(x_d):
                tiles = []
                for dt in range(NDT):
                    t = xtp.tile([P, S], F16, tag="xt")
                    nc.sync.dma_start(
                        t[:], x_d[:, dt * P:(dt + 1) * P], transpose=True)
                    tiles.append(t)
                return tiles

            xt_k = load_xt(xk_d)
            xt_v = load_xt(xv_d)
            xt_q = load_xt(xq_d)

            # ---- feature-major projection: out^T[ds, t] (Q^T / K^T) ----
            def proj_T(dst, w_t, b_t, xt, tc_idx):
                ts0 = tc_idx * QCH
                for dot in range(2):
                    ps = psA.tile([P, QCH], F32, tag="proj")
                    for dt in range(NDT):
                        nc.tensor.matmul(
                            ps[:],
                            w_t[:, dt, dot * P:(dot + 1) * P],
                            xt[dt][:, ts0:ts0 + QCH],
                            start=(dt == 0), stop=(dt == NDT - 1),
                        )
                    nc.scalar.activation(
                        dst[:, dot, ts0:ts0 + QCH], ps[:], AF.Identity,
                        bias=b_t[:, dot, :])

            for tci in range(NQC):
                proj_T(kt_t, wk_t, bk_t, xt_k, tci)

            # ---- token-major V projection ----
            for tt in range(NTT):
                ps = psA.tile([P, DS], F32, tag="proj")
                for dt in range(NDT):
                    nc.tensor.matmul(
                        ps[:],
                        xt_v[dt][:, tt * P:(tt + 1) * P],
                        wv_t[:, dt, :],
                        start=(dt == 0), stop=(dt == NDT - 1),
                    )
                nc.vector.tensor_add(v_t[:, tt, :], ps[:], bv_b[:, :])

            proj_T(qt_t, wq_t, bq_t, xt_q, 0)

            # ---- attention + output projection, per q-chunk ----
            for qc in range(NQC):
                qs = qc * QCH
                for pr in range(2):
                    acc = psB.tile([P, QCH], F32, tag="acc")
                    sm = psB.tile([P, QCH], F32, tag="sum")
                    for kt in range(NKT):
                        ks = kt * P
                        sc0 = psB.tile([P, QCH], F32, tag="sc")
                        sc1 = psB.tile([P, QCH], F32, tag="sc")
                        # scores^T, two heads row-packed (K=64 each)
                        nc.tensor.matmul(
                            sc0[:], kt_t[0:64, pr, ks:ks + P],
                            qt_t[0:64, pr, qs:qs + QCH],
                            start=True, stop=True)
                        nc.tensor.matmul(
                            sc1[:], kt_t[64:128, pr, ks:ks + P],
                            qt_t[64:128, pr, qs:qs + QCH],
                            start=True, stop=True)
                        e0 = expp.tile([P, QCH], F16, tag="exp")
                        e1 = expp.tile([P, QCH], F16, tag="exp")
                        nc.scalar.activation(e0[:], sc0[:], AF.Exp, scale=SCALE)
                        nc.scalar.activation(e1[:], sc1[:], AF.Exp, scale=SCALE)
                        h0 = 2 * pr
                        h1 = 2 * pr + 1
                        nc.tensor.matmul(
                            acc[0:64, :], v_t[:, kt, h0 * DH:(h0 + 1) * DH],
                            e0[:], start=(kt == 0), stop=(kt == NKT - 1),
                            tile_position=(0, 0), skip_group_check=True)
                        nc.tensor.matmul(
                            acc[64:128, :], v_t[:, kt, h1 * DH:(h1 + 1) * DH],
                            e1[:], start=(kt == 0), stop=(kt == NKT - 1),
                            tile_position=(0, 64), skip_group_check=True)
                        nc.tensor.matmul(
                            sm[0:64, :], ones_t[:], e0[:],
                            start=(kt == 0), stop=(kt == NKT - 1),
                            tile_position=(0, 0), skip_group_check=True)
                        nc.tensor.matmul(
                            sm[64:128, :], ones_t[:], e1[:],
                            start=(kt == 0), stop=(kt == NKT - 1),
                            tile_position=(0, 64), skip_group_check=True)
                    rc = rcpp.tile([P, QCH], F32, tag="rcp")
                    nc.vector.reciprocal(rc[:], sm[:])
                    nc.vector.tensor_mul(an_t[:, pr, qs:qs + QCH], acc[:], rc[:])

                if qc + 1 < NQC:
                    proj_T(qt_t, wq_t, bq_t, xt_q, qc + 1)

                # output projection for this q-chunk's token tiles
                for tt4 in range(QCH // P):
                    tt = qc * (QCH // P) + tt4
                    for half in range(2):
                        po = psB.tile([P, QCH], F32, tag="sc")
                        for pr in range(2):
                            nc.tensor.matmul(
                                po[:],
                                an_t[:, pr, tt * P:(tt + 1) * P],
                                wo_t[:, pr, half * QCH:(half + 1) * QCH],
                                start=(pr == 0), stop=(pr == 1))
                        ob = osbp.tile([P, QCH], F32, tag="osb")
                        nc.vector.tensor_add(
                            ob[:], po[:], bo_b[:, half * QCH:(half + 1) * QCH])
                        nc.sync.dma_start(
                            partial_d[tt * P:(tt + 1) * P,
                                      half * QCH:(half + 1) * QCH],
                            ob[:])

            # ---- reduce partials within the batch group ----
            nc.gpsimd.collective_compute(
                "ReduceScatter",
                mybir.AluOpType.add,
                replica_groups=REPLICA_GROUPS,
                ins=[partial_d[:]],
                outs=[rs_d[:]],
            )
            nc.sync.dma_start(out_d[:], rs_d[:])

    nc.compile()
    return nc


def _get_nc():
    global _CACHED_NC
    if _CACHED_NC is None:
        _CACHED_NC = _build_module()
    return _CACHED_NC


def _make_in_maps(query, key, value, Wq, bq, Wk, bk, Wv, bv, Wo, bo):
    query = np.asarray(query, dtype=np.float32)
    key = np.asarray(key, dtype=np.float32)
    value = np.asarray(value, dtype=np.float32)
    Wq = np.asarray(Wq, dtype=np.float32)
    Wk = np.asarray(Wk, dtype=np.float32)
    Wv = np.asarray(Wv, dtype=np.float32)
    Wo = np.asarray(Wo, dtype=np.float32)
    bq = np.asarray(bq, dtype=np.float32)
    bk = np.asarray(bk, dtype=np.float32)
    bv = np.asarray(bv, dtype=np.float32)
    bo = np.asarray(bo, dtype=np.float32)

    in_maps = []
    for c in range(NCORES):
        b = c // GPB
        g = c % GPB
        sl = slice(g * DS, (g + 1) * DS)
        in_maps.append({
            "xq": query[b].astype(np.float16),
            "xk": key[b].astype(np.float16),
            "xv": value[b].astype(np.float16),
            "wq": Wq[:, sl].astype(np.float16),
            "wk": Wk[:, sl].astype(np.float16),
            "wv": Wv[:, sl].astype(np.float16),
            "wo": Wo[sl, :].astype(np.float16),
            "bq": bq[sl].reshape(DS, 1).copy(),
            "bk": bk[sl].reshape(DS, 1).copy(),
            "bv": bv[sl].reshape(1, DS).copy(),
            "bo": (bo if g == 0 else np.zeros_like(bo)).reshape(1, D).copy(),
        })
    return in_maps


def run(inputs, trace=False, trace_cores=None):
    """Run the SPMD kernel; returns (full_output, BassKernelResults)."""
    nc = _get_nc()
    in_maps = _make_in_maps(**inputs)
    res = run_bass_kernel_spmd(
        nc, in_maps, core_ids=list(range(NCORES)), trace=trace,
        trace_cores=trace_cores)
    out = np.empty((B, S, D), dtype=np.float32)
    for c in range(NCORES):
        b = c // GPB
        g = c % GPB
        out[b, g * (S // GPB):(g + 1) * (S // GPB), :] = res.results[c]["out"]
    return out, res


def kernel(**inputs):
    out, _ = run(inputs, trace=False)
    return out


# revision 6
# speedup vs baseline: 1.4422x; 1.4422x over previous
"""Multi-head attention (B=2, S=2048, D=1024, H=16) on 8 Trainium2 NeuronCores.

Sharding: batch x head-group. Core c handles batch b = c//4 and heads
[4*(c%4), 4*(c%4)+4) (a 256-wide slice of the QKV projection output and the
matching 256-row slice of Wo). Each core computes its partial output
projection; a 4-way ReduceScatter per batch group sums the partials and
leaves each core with a [512, 1024] row block of the final output, which the
host reassembles.

Per-core dataflow (all matmul operands fp16, fp32 PSUM accumulation):
  - x^T tiles arrive via hardware DMA-transpose (fp16).
  - Q^T, K^T feature-major [256, 2048]; V token-major [2048, 256].
  - Scores computed transposed (S^T[k, q] = K_h @ Q_h^T) with two heads
    row-packed in the PE array; softmax without max-subtraction (exp via
    ScalarE with the 1/sqrt(dh) scale folded in); per-q sums via a
    ones-matmul col-packed two heads at a time, broadcast across partitions
    for free; attn@V col-packed two heads at a time.
  - Output projection from attn_norm^T with the head-pair dim as the
    contraction tiles.
"""

import numpy as np

import concourse.bass as bass  # noqa: F401  (engine namespaces via nc)
import concourse.mybir as mybir
import concourse.tile as tile
from concourse import bacc
from concourse.bass_utils import run_bass_kernel_spmd

F32 = mybir.dt.float32
F16 = mybir.dt.float16
AF = mybir.ActivationFunctionType

B, S, D = 2, 2048, 1024
H, DH = 16, 64
NCORES = 8
GPB = 4                # cores per batch group
HPC = H // GPB         # heads per core
DS = HPC * DH          # 256: per-core slice of the projection output
P = 128
NDT = D // P           # 8 d_model tiles
NTT = S // P           # 16 token tiles
QCH = 512              # q-chunk (PSUM bank = 512 fp32)
NQC = S // QCH         # 4
NKT = S // P           # 16 k tiles
SCALE = float(1.0 / np.sqrt(DH))

REPLICA_GROUPS = [[0, 1, 2, 3], [4, 5, 6, 7]]

_CACHED_NC = None


def _build_module():
    nc = bacc.Bacc("TRN2", target_bir_lowering=False, debug=False,
                   num_devices=NCORES)

    xq_d = nc.dram_tensor("xq", [S, D], F16, kind="ExternalInput")
    xk_d = nc.dram_tensor("xk", [S, D], F16, kind="ExternalInput")
    xv_d = nc.dram_tensor("xv", [S, D], F16, kind="ExternalInput")
    wq_d = nc.dram_tensor("wq", [D, DS], F16, kind="ExternalInput")
    wk_d = nc.dram_tensor("wk", [D, DS], F16, kind="ExternalInput")
    wv_d = nc.dram_tensor("wv", [D, DS], F16, kind="ExternalInput")
    wo_d = nc.dram_tensor("wo", [DS, D], F16, kind="ExternalInput")
    bq_d = nc.dram_tensor("bq", [DS, 1], F32, kind="ExternalInput")
    bk_d = nc.dram_tensor("bk", [DS, 1], F32, kind="ExternalInput")
    bv_d = nc.dram_tensor("bv", [1, DS], F32, kind="ExternalInput")
    bo_d = nc.dram_tensor("bo", [1, D], F32, kind="ExternalInput")

    out_d = nc.dram_tensor("out", [S // GPB, D], F32, kind="ExternalOutput")
    partial_d = nc.dram_tensor("partial", [S, D], F32)
    rs_d = nc.dram_tensor("rs_out", [S // GPB, D], F32)

    with tile.TileContext(nc) as tc:
        with (
            tc.tile_pool(name="cst", bufs=1) as cst,
            tc.tile_pool(name="xt", bufs=10) as xtp,
            tc.tile_pool(name="exp", bufs=18) as expp,
            tc.tile_pool(name="rcp", bufs=2) as rcpp,
            tc.tile_pool(name="osb", bufs=2) as osbp,
            tc.tile_pool(name="psA", bufs=2, space="PSUM") as psA,
            tc.tile_pool(name="psB", bufs=2, space="PSUM") as psB,
            tc.tile_pool(name="psC", bufs=1, space="PSUM") as psC,
        ):
            # ---- constants ----
            wq_t = cst.tile([P, NDT, DS], F16, tag="wq")
            wk_t = cst.tile([P, NDT, DS], F16, tag="wk")
            wv_t = cst.tile([P, NDT, DS], F16, tag="wv")
            wo_t = cst.tile([P, 2, D], F16, tag="wo")
            nc.sync.dma_start(wq_t[:], wq_d.rearrange("(a p) n -> p a n", p=P))
            nc.sync.dma_start(wk_t[:], wk_d.rearrange("(a p) n -> p a n", p=P))
            nc.sync.dma_start(wv_t[:], wv_d.rearrange("(a p) n -> p a n", p=P))
            nc.sync.dma_start(wo_t[:], wo_d.rearrange("(a p) n -> p a n", p=P))

            bq_t = cst.tile([P, 2, 1], F32, tag="bq")
            bk_t = cst.tile([P, 2, 1], F32, tag="bk")
            nc.sync.dma_start(bq_t[:], bq_d.rearrange("(a p) o -> p a o", p=P))
            nc.sync.dma_start(bk_t[:], bk_d.rearrange("(a p) o -> p a o", p=P))

            bv_row = cst.tile([1, DS], F32, tag="bvr")
            bo_row = cst.tile([1, D], F32, tag="bor")
            nc.sync.dma_start(bv_row[:], bv_d[:])
            nc.sync.dma_start(bo_row[:], bo_d[:])
            bv_b = cst.tile([P, DS], F32, tag="bvb")
            bo_b = cst.tile([P, D], F32, tag="bob")
            nc.gpsimd.partition_broadcast(bv_b[:], bv_row[:])
            nc.gpsimd.partition_broadcast(bo_b[:], bo_row[:])

            ones_t = cst.tile([P, DH], F16, tag="ones")
            nc.vector.memset(ones_t[:], 1.0)

            # ---- activations: resident projections ----
            qt_t = cst.tile([P, 2, S], F16, tag="qt")   # Q^T  (pair, t)
            kt_t = cst.tile([P, 2, S], F16, tag="kt")   # K^T
            v_t = cst.tile([P, NTT, DS], F16, tag="vt")  # V token-major
            an_t = cst.tile([P, 2, S], F16, tag="an")   # attn_norm^T

            # ---- transposed input tiles (DMA transpose, fp16) ----
            def load_xt(x_d):
                tiles = []
                for dt in range(NDT):
                    t = xtp.tile([P, S], F16, tag="xt")
                    nc.sync.dma_start(
                        t[:], x_d[:, dt * P:(dt + 1) * P], transpose=True)
                    tiles.append(t)
                return tiles

            xt_k = load_xt(xk_d)
            xt_v = load_xt(xv_d)
            xt_q = load_xt(xq_d)

            # ---- feature-major projection: out^T[ds, t] (Q^T / K^T) ----
            def proj_T(dst, w_t, b_t, xt, tc_idx):
                ts0 = tc_idx * QCH
                for dot in range(2):
                    ps = psA.tile([P, QCH], F32, tag="proj")
                    for dt in range(NDT):
                        nc.tensor.matmul(
                            ps[:],
                            w_t[:, dt, dot * P:(dot + 1) * P],
                            xt[dt][:, ts0:ts0 + QCH],
                            start=(dt == 0), stop=(dt == NDT - 1),
                        )
                    nc.scalar.activation(
                        dst[:, dot, ts0:ts0 + QCH], ps[:], AF.Identity,
                        bias=b_t[:, dot, :])

            for tci in range(NQC):
                proj_T(kt_t, wk_t, bk_t, xt_k, tci)

            # ---- token-major V projection ----
            for tt in range(NTT):
                ps = psA.tile([P, DS], F32, tag="proj")
                for dt in range(NDT):
                    nc.tensor.matmul(
                        ps[:],
                        xt_v[dt][:, tt * P:(tt + 1) * P],
                        wv_t[:, dt, :],
                        start=(dt == 0), stop=(dt == NDT - 1),
                    )
                nc.vector.tensor_add(v_t[:, tt, :], ps[:], bv_b[:, :])

            proj_T(qt_t, wq_t, bq_t, xt_q, 0)

            # ---- attention + output projection, per q-chunk ----
            # Phase-separated so the PE array tiling mode stays constant
            # across long instruction runs (mode switches drain the array):
            # phase 1 = all 16 k-tiles of scores (64x128 row tiling),
            # phase 2 = all attn@V + sums (128x64 col tiling).
            for qc in range(NQC):
                qs = qc * QCH
                for pr in range(2):
                    # phase 1: scores^T for all k-tiles, two heads row-packed
                    # (h0 -> PE rows 0-63, h1 -> rows 64-127, concurrent).
                    # Each psum tile spans 2 banks = scores for 2 k-tiles, so
                    # exp amortizes its per-instruction overhead over 1024
                    # elements.
                    etiles = []  # [kp] -> (e_h0, e_h1), each [P, 2*QCH] fp16
                    for kp in range(NKT // 2):
                        sc0 = psB.tile([P, 2 * QCH], F32, tag="sc")
                        sc1 = psB.tile([P, 2 * QCH], F32, tag="sc")
                        for j in range(2):
                            ks = (2 * kp + j) * P
                            col = slice(j * QCH, (j + 1) * QCH)
                            nc.tensor.matmul(
                                sc0[:, col], kt_t[0:64, pr, ks:ks + P],
                                qt_t[0:64, pr, qs:qs + QCH],
                                start=True, stop=True)
                            nc.tensor.matmul(
                                sc1[:, col], kt_t[64:128, pr, ks:ks + P],
                                qt_t[64:128, pr, qs:qs + QCH],
                                start=True, stop=True)
                        e0 = expp.tile([P, 2 * QCH], F16, tag="exp")
                        e1 = expp.tile([P, 2 * QCH], F16, tag="exp")
                        nc.scalar.activation(e0[:], sc0[:], AF.Exp, scale=SCALE)
                        nc.scalar.activation(e1[:], sc1[:], AF.Exp, scale=SCALE)
                        etiles.append((e0, e1))

                    # phase 2: attn@V + sums, two heads col-packed
                    # (h0 -> PSUM partitions 0-63, h1 -> 64-127, concurrent).
                    h0 = 2 * pr
                    h1 = 2 * pr + 1
                    acc = psC.tile([P, QCH], F32, tag="acc")
                    sm = psC.tile([P, QCH], F32, tag="sum")
                    for kt in range(NKT):
                        e0, e1 = etiles[kt // 2]
                        col = slice((kt % 2) * QCH, (kt % 2 + 1) * QCH)
                        st = (kt == 0)
                        sp = (kt == NKT - 1)
                        nc.tensor.matmul(
                            acc[0:64, :], v_t[:, kt, h0 * DH:(h0 + 1) * DH],
                            e0[:, col], start=st, stop=sp,
                            tile_position=(0, 0), skip_group_check=True)
                        nc.tensor.matmul(
                            acc[64:128, :], v_t[:, kt, h1 * DH:(h1 + 1) * DH],
                            e1[:, col], start=st, stop=sp,
                            tile_position=(0, 64), skip_group_check=True)
                        nc.tensor.matmul(
                            sm[0:64, :], ones_t[:], e0[:, col],
                            start=st, stop=sp,
                            tile_position=(0, 0), skip_group_check=True)
                        nc.tensor.matmul(
                            sm[64:128, :], ones_t[:], e1[:, col],
                            start=st, stop=sp,
                            tile_position=(0, 64), skip_group_check=True)
                    rc = rcpp.tile([P, QCH], F32, tag="rcp")
                    nc.vector.reciprocal(rc[:], sm[:])
                    nc.vector.tensor_mul(an_t[:, pr, qs:qs + QCH], acc[:], rc[:])

                if qc + 1 < NQC:
                    proj_T(qt_t, wq_t, bq_t, xt_q, qc + 1)

                # output projection for this q-chunk's token tiles
                for tt4 in range(QCH // P):
                    tt = qc * (QCH // P) + tt4
                    po = psB.tile([P, 2 * QCH], F32, tag="sc")
                    for half in range(2):
                        for pr in range(2):
                            nc.tensor.matmul(
                                po[:, half * QCH:(half + 1) * QCH],
                                an_t[:, pr, tt * P:(tt + 1) * P],
                                wo_t[:, pr, half * QCH:(half + 1) * QCH],
                                start=(pr == 0), stop=(pr == 1))
                    ob = osbp.tile([P, D], F32, tag="osb")
                    nc.vector.tensor_add(ob[:], po[:], bo_b[:])
                    nc.sync.dma_start(
                        partial_d[tt * P:(tt + 1) * P, :], ob[:])

                # overlapped reduce of this q-chunk's partial rows
                nc.gpsimd.collective_compute(
                    "ReduceScatter",
                    mybir.AluOpType.add,
                    replica_groups=REPLICA_GROUPS,
                    ins=[partial_d[qs:qs + QCH, :]],
                    outs=[rs_d[qc * P:(qc + 1) * P, :]],
                )

            nc.sync.dma_start(out_d[:], rs_d[:])

    nc.compile()
    return nc


def _get_nc():
    global _CACHED_NC
    if _CACHED_NC is None:
        _CACHED_NC = _build_module()
    return _CACHED_NC


def _make_in_maps(query, key, value, Wq, bq, Wk, bk, Wv, bv, Wo, bo):
    query = np.asarray(query, dtype=np.float32)
    key = np.asarray(key, dtype=np.float32)
    value = np.asarray(value, dtype=np.float32)
    Wq = np.asarray(Wq, dtype=np.float32)
    Wk = np.asarray(Wk, dtype=np.float32)
    Wv = np.asarray(Wv, dtype=np.float32)
    Wo = np.asarray(Wo, dtype=np.float32)
    bq = np.asarray(bq, dtype=np.float32)
    bk = np.asarray(bk, dtype=np.float32)
    bv = np.asarray(bv, dtype=np.float32)
    bo = np.asarray(bo, dtype=np.float32)

    in_maps = []
    for c in range(NCORES):
        b = c // GPB
        g = c % GPB
        sl = slice(g * DS, (g + 1) * DS)
        in_maps.append({
            "xq": query[b].astype(np.float16),
            "xk": key[b].astype(np.float16),
            "xv": value[b].astype(np.float16),
            "wq": Wq[:, sl].astype(np.float16),
            "wk": Wk[:, sl].astype(np.float16),
            "wv": Wv[:, sl].astype(np.float16),
            "wo": Wo[sl, :].astype(np.float16),
            "bq": bq[sl].reshape(DS, 1).copy(),
            "bk": bk[sl].reshape(DS, 1).copy(),
            "bv": bv[sl].reshape(1, DS).copy(),
            "bo": (bo if g == 0 else np.zeros_like(bo)).reshape(1, D).copy(),
        })
    return in_maps


def run(inputs, trace=False, trace_cores=None):
    """Run the SPMD kernel; returns (full_output, BassKernelResults)."""
    nc = _get_nc()
    in_maps = _make_in_maps(**inputs)
    res = run_bass_kernel_spmd(
        nc, in_maps, core_ids=list(range(NCORES)), trace=trace,
        trace_cores=trace_cores)
    out = np.empty((B, S, D), dtype=np.float32)
    for c in range(NCORES):
        b = c // GPB
        g = c % GPB
        o = res.results[c]["out"]
        for qc in range(NQC):
            out[b, qc * QCH + g * P:qc * QCH + (g + 1) * P, :] = \
                o[qc * P:(qc + 1) * P, :]
    return out, res


def kernel(**inputs):
    out, _ = run(inputs, trace=False)
    return out


# revision 7
# speedup vs baseline: 1.4925x; 1.0349x over previous
"""Multi-head attention (B=2, S=2048, D=1024, H=16) on 8 Trainium2 NeuronCores.

Sharding: batch x head-group. Core c handles batch b = c//4 and heads
[4*(c%4), 4*(c%4)+4) (a 256-wide slice of the QKV projection output and the
matching 256-row slice of Wo). Each core computes its partial output
projection; a 4-way ReduceScatter per batch group sums the partials and
leaves each core with a [512, 1024] row block of the final output, which the
host reassembles.

Per-core dataflow (all matmul operands fp16, fp32 PSUM accumulation):
  - x^T tiles arrive via hardware DMA-transpose (fp16).
  - Q^T, K^T feature-major [256, 2048]; V token-major [2048, 256].
  - Scores computed transposed (S^T[k, q] = K_h @ Q_h^T) with two heads
    row-packed in the PE array; softmax without max-subtraction (exp via
    ScalarE with the 1/sqrt(dh) scale folded in); per-q sums via a
    ones-matmul col-packed two heads at a time, broadcast across partitions
    for free; attn@V col-packed two heads at a time.
  - Output projection from attn_norm^T with the head-pair dim as the
    contraction tiles.
"""

import numpy as np

import concourse.bass as bass  # noqa: F401  (engine namespaces via nc)
import concourse.mybir as mybir
import concourse.tile as tile
from concourse import bacc
from concourse.bass_utils import run_bass_kernel_spmd

F32 = mybir.dt.float32
F16 = mybir.dt.float16
AF = mybir.ActivationFunctionType

B, S, D = 2, 2048, 1024
H, DH = 16, 64
NCORES = 8
GPB = 4                # cores per batch group
HPC = H // GPB         # heads per core
DS = HPC * DH          # 256: per-core slice of the projection output
P = 128
NDT = D // P           # 8 d_model tiles
NTT = S // P           # 16 token tiles
QCH = 512              # q-chunk (PSUM bank = 512 fp32)
NQC = S // QCH         # 4
NKT = S // P           # 16 k tiles
SCALE = float(1.0 / np.sqrt(DH))

REPLICA_GROUPS = [[0, 1, 2, 3], [4, 5, 6, 7]]

_CACHED_NC = None


def _build_module():
    nc = bacc.Bacc("TRN2", target_bir_lowering=False, debug=False,
                   num_devices=NCORES)

    xq_d = nc.dram_tensor("xq", [S, D], F16, kind="ExternalInput")
    xk_d = nc.dram_tensor("xk", [S, D], F16, kind="ExternalInput")
    xv_d = nc.dram_tensor("xv", [S, D], F16, kind="ExternalInput")
    wq_d = nc.dram_tensor("wq", [D, DS], F16, kind="ExternalInput")
    wk_d = nc.dram_tensor("wk", [D, DS], F16, kind="ExternalInput")
    wv_d = nc.dram_tensor("wv", [D, DS], F16, kind="ExternalInput")
    wo_d = nc.dram_tensor("wo", [DS, D], F16, kind="ExternalInput")
    bq_d = nc.dram_tensor("bq", [DS, 1], F32, kind="ExternalInput")
    bk_d = nc.dram_tensor("bk", [DS, 1], F32, kind="ExternalInput")
    bv_d = nc.dram_tensor("bv", [1, DS], F32, kind="ExternalInput")
    bo_d = nc.dram_tensor("bo", [1, D], F32, kind="ExternalInput")

    out_d = nc.dram_tensor("out", [S // GPB, D], F32, kind="ExternalOutput")
    partial_d = nc.dram_tensor("partial", [S, D], F32)
    rs_d = nc.dram_tensor("rs_out", [S // GPB, D], F32)

    with tile.TileContext(nc) as tc:
        with (
            tc.tile_pool(name="cst", bufs=1) as cst,
            tc.tile_pool(name="xt", bufs=10) as xtp,
            tc.tile_pool(name="exp", bufs=22) as expp,
            tc.tile_pool(name="rcp", bufs=2) as rcpp,
            tc.tile_pool(name="osb", bufs=2) as osbp,
            tc.tile_pool(name="psB", bufs=3, space="PSUM") as psB,
            tc.tile_pool(name="psC", bufs=1, space="PSUM") as psC,
        ):
            # ---- constants ----
            wq_t = cst.tile([P, NDT, DS], F16, tag="wq")
            wk_t = cst.tile([P, NDT, DS], F16, tag="wk")
            wv_t = cst.tile([P, NDT, DS], F16, tag="wv")
            wo_t = cst.tile([P, 2, D], F16, tag="wo")
            nc.sync.dma_start(wq_t[:], wq_d.rearrange("(a p) n -> p a n", p=P))
            nc.sync.dma_start(wk_t[:], wk_d.rearrange("(a p) n -> p a n", p=P))
            nc.sync.dma_start(wv_t[:], wv_d.rearrange("(a p) n -> p a n", p=P))
            nc.sync.dma_start(wo_t[:], wo_d.rearrange("(a p) n -> p a n", p=P))

            bq_t = cst.tile([P, 2, 1], F32, tag="bq")
            bk_t = cst.tile([P, 2, 1], F32, tag="bk")
            nc.sync.dma_start(bq_t[:], bq_d.rearrange("(a p) o -> p a o", p=P))
            nc.sync.dma_start(bk_t[:], bk_d.rearrange("(a p) o -> p a o", p=P))

            bv_row = cst.tile([1, DS], F32, tag="bvr")
            bo_row = cst.tile([1, D], F32, tag="bor")
            nc.sync.dma_start(bv_row[:], bv_d[:])
            nc.sync.dma_start(bo_row[:], bo_d[:])
            bv_b = cst.tile([P, DS], F32, tag="bvb")
            bo_b = cst.tile([P, D], F32, tag="bob")
            nc.gpsimd.partition_broadcast(bv_b[:], bv_row[:])
            nc.gpsimd.partition_broadcast(bo_b[:], bo_row[:])

            ones_t = cst.tile([P, DH], F16, tag="ones")
            nc.vector.memset(ones_t[:], 1.0)

            # ---- activations: resident projections ----
            qt_t = cst.tile([P, 2, S], F16, tag="qt")   # Q^T  (pair, t)
            kt_t = cst.tile([P, 2, S], F16, tag="kt")   # K^T
            v_t = cst.tile([P, NTT, DS], F16, tag="vt")  # V token-major
            an_t = cst.tile([P, 2, S], F16, tag="an")   # attn_norm^T

            # ---- transposed input tiles (DMA transpose, fp16) ----
            def load_xt(x_d):
                tiles = []
                for dt in range(NDT):
                    t = xtp.tile([P, S], F16, tag="xt")
                    nc.sync.dma_start(
                        t[:], x_d[:, dt * P:(dt + 1) * P], transpose=True)
                    tiles.append(t)
                return tiles

            xt_k = load_xt(xk_d)
            xt_v = load_xt(xv_d)
            xt_q = load_xt(xq_d)

            # ---- feature-major projection: out^T[ds, t] (Q^T / K^T) ----
            def proj_T(dst, w_t, b_t, xt, tc_idx):
                ts0 = tc_idx * QCH
                ps = psB.tile([P, 2 * QCH], F32, tag="sc")
                for dot in range(2):
                    col = slice(dot * QCH, (dot + 1) * QCH)
                    for dt in range(NDT):
                        nc.tensor.matmul(
                            ps[:, col],
                            w_t[:, dt, dot * P:(dot + 1) * P],
                            xt[dt][:, ts0:ts0 + QCH],
                            start=(dt == 0), stop=(dt == NDT - 1),
                        )
                for dot in range(2):
                    nc.scalar.activation(
                        dst[:, dot, ts0:ts0 + QCH],
                        ps[:, dot * QCH:(dot + 1) * QCH], AF.Identity,
                        bias=b_t[:, dot, :])

            for tci in range(NQC):
                proj_T(kt_t, wk_t, bk_t, xt_k, tci)

            # ---- token-major V projection ----
            for tt in range(NTT):
                ps = psB.tile([P, DS], F32, tag="sc")
                for dt in range(NDT):
                    nc.tensor.matmul(
                        ps[:],
                        xt_v[dt][:, tt * P:(tt + 1) * P],
                        wv_t[:, dt, :],
                        start=(dt == 0), stop=(dt == NDT - 1),
                    )
                nc.vector.tensor_add(v_t[:, tt, :], ps[:], bv_b[:, :])

            proj_T(qt_t, wq_t, bq_t, xt_q, 0)

            # ---- attention + output projection, per q-chunk ----
            # Phase-separated so the PE array tiling mode stays constant
            # across long instruction runs (mode switches drain the array):
            # phase 1 = all 16 k-tiles of scores (64x128 row tiling),
            # phase 2 = all attn@V + sums (128x64 col tiling).
            for qc in range(NQC):
                qs = qc * QCH
                for pr in range(2):
                    # phase 1: scores^T for all k-tiles, two heads row-packed
                    # (h0 -> PE rows 0-63, h1 -> rows 64-127, concurrent).
                    # Each psum tile spans 2 banks = scores for 2 k-tiles, so
                    # exp amortizes its per-instruction overhead over 1024
                    # elements.
                    etiles = []  # [kp] -> (e_h0, e_h1), each [P, 2*QCH] fp16
                    for kp in range(NKT // 2):
                        sc0 = psB.tile([P, 2 * QCH], F32, tag="sc")
                        sc1 = psB.tile([P, 2 * QCH], F32, tag="sc")
                        for j in range(2):
                            ks = (2 * kp + j) * P
                            col = slice(j * QCH, (j + 1) * QCH)
                            nc.tensor.matmul(
                                sc0[:, col], kt_t[0:64, pr, ks:ks + P],
                                qt_t[0:64, pr, qs:qs + QCH],
                                start=True, stop=True)
                            nc.tensor.matmul(
                                sc1[:, col], kt_t[64:128, pr, ks:ks + P],
                                qt_t[64:128, pr, qs:qs + QCH],
                                start=True, stop=True)
                        e0 = expp.tile([P, 2 * QCH], F16, tag="exp")
                        e1 = expp.tile([P, 2 * QCH], F16, tag="exp")
                        nc.scalar.activation(e0[:], sc0[:], AF.Exp, scale=SCALE)
                        nc.scalar.activation(e1[:], sc1[:], AF.Exp, scale=SCALE)
                        etiles.append((e0, e1))

                    # phase 2: attn@V + sums, two heads col-packed
                    # (h0 -> PSUM partitions 0-63, h1 -> 64-127, concurrent).
                    h0 = 2 * pr
                    h1 = 2 * pr + 1
                    acc = psC.tile([P, QCH], F32, tag="acc")
                    sm = psC.tile([P, QCH], F32, tag="sum")
                    for kt in range(NKT):
                        e0, e1 = etiles[kt // 2]
                        col = slice((kt % 2) * QCH, (kt % 2 + 1) * QCH)
                        st = (kt == 0)
                        sp = (kt == NKT - 1)
                        nc.tensor.matmul(
                            acc[0:64, :], v_t[:, kt, h0 * DH:(h0 + 1) * DH],
                            e0[:, col], start=st, stop=sp,
                            tile_position=(0, 0), skip_group_check=True)
                        nc.tensor.matmul(
                            acc[64:128, :], v_t[:, kt, h1 * DH:(h1 + 1) * DH],
                            e1[:, col], start=st, stop=sp,
                            tile_position=(0, 64), skip_group_check=True)
                        nc.tensor.matmul(
                            sm[0:64, :], ones_t[:], e0[:, col],
                            start=st, stop=sp,
                            tile_position=(0, 0), skip_group_check=True)
                        nc.tensor.matmul(
                            sm[64:128, :], ones_t[:], e1[:, col],
                            start=st, stop=sp,
                            tile_position=(0, 64), skip_group_check=True)
                    rc = rcpp.tile([P, QCH], F32, tag="rcp")
                    nc.vector.reciprocal(rc[:], sm[:])
                    nc.vector.tensor_mul(an_t[:, pr, qs:qs + QCH], acc[:], rc[:])

                if qc + 1 < NQC:
                    proj_T(qt_t, wq_t, bq_t, xt_q, qc + 1)

                # output projection for this q-chunk's token tiles
                for tt4 in range(QCH // P):
                    tt = qc * (QCH // P) + tt4
                    po = psB.tile([P, 2 * QCH], F32, tag="sc")
                    for half in range(2):
                        for pr in range(2):
                            nc.tensor.matmul(
                                po[:, half * QCH:(half + 1) * QCH],
                                an_t[:, pr, tt * P:(tt + 1) * P],
                                wo_t[:, pr, half * QCH:(half + 1) * QCH],
                                start=(pr == 0), stop=(pr == 1))
                    ob = osbp.tile([P, D], F32, tag="osb")
                    nc.vector.tensor_add(ob[:], po[:], bo_b[:])
                    nc.sync.dma_start(
                        partial_d[tt * P:(tt + 1) * P, :], ob[:])

                # overlapped reduce of this q-chunk's partial rows
                for half in range(2):
                    j = 2 * qc + half
                    nc.gpsimd.collective_compute(
                        "ReduceScatter",
                        mybir.AluOpType.add,
                        replica_groups=REPLICA_GROUPS,
                        ins=[partial_d[j * 256:(j + 1) * 256, :]],
                        outs=[rs_d[j * 64:(j + 1) * 64, :]],
                    )
                    nc.sync.dma_start(out_d[j * 64:(j + 1) * 64, :],
                                      rs_d[j * 64:(j + 1) * 64, :])

    nc.compile()
    return nc


def _get_nc():
    global _CACHED_NC
    if _CACHED_NC is None:
        _CACHED_NC = _build_module()
    return _CACHED_NC


def _make_in_maps(query, key, value, Wq, bq, Wk, bk, Wv, bv, Wo, bo):
    query = np.asarray(query, dtype=np.float32)
    key = np.asarray(key, dtype=np.float32)
    value = np.asarray(value, dtype=np.float32)
    Wq = np.asarray(Wq, dtype=np.float32)
    Wk = np.asarray(Wk, dtype=np.float32)
    Wv = np.asarray(Wv, dtype=np.float32)
    Wo = np.asarray(Wo, dtype=np.float32)
    bq = np.asarray(bq, dtype=np.float32)
    bk = np.asarray(bk, dtype=np.float32)
    bv = np.asarray(bv, dtype=np.float32)
    bo = np.asarray(bo, dtype=np.float32)

    in_maps = []
    for c in range(NCORES):
        b = c // GPB
        g = c % GPB
        sl = slice(g * DS, (g + 1) * DS)
        in_maps.append({
            "xq": query[b].astype(np.float16),
            "xk": key[b].astype(np.float16),
            "xv": value[b].astype(np.float16),
            "wq": Wq[:, sl].astype(np.float16),
            "wk": Wk[:, sl].astype(np.float16),
            "wv": Wv[:, sl].astype(np.float16),
            "wo": Wo[sl, :].astype(np.float16),
            "bq": bq[sl].reshape(DS, 1).copy(),
            "bk": bk[sl].reshape(DS, 1).copy(),
            "bv": bv[sl].reshape(1, DS).copy(),
            "bo": (bo if g == 0 else np.zeros_like(bo)).reshape(1, D).copy(),
        })
    return in_maps


def run(inputs, trace=False, trace_cores=None):
    """Run the SPMD kernel; returns (full_output, BassKernelResults)."""
    nc = _get_nc()
    in_maps = _make_in_maps(**inputs)
    res = run_bass_kernel_spmd(
        nc, in_maps, core_ids=list(range(NCORES)), trace=trace,
        trace_cores=trace_cores)
    out = np.empty((B, S, D), dtype=np.float32)
    for c in range(NCORES):
        b = c // GPB
        g = c % GPB
        o = res.results[c]["out"]
        for j in range(8):
            out[b, j * 256 + g * 64:j * 256 + (g + 1) * 64, :] = \
                o[j * 64:(j + 1) * 64, :]
    return out, res


def kernel(**inputs):
    out, _ = run(inputs, trace=False)
    return out
